# revision 33
# baseline (speedup 1.0000x reference)
"""Trainium2 Bass kernel for 16-head causal MHA (B=4, S=2048, E=1024, D=64).

Sharding: 8 cores = 4 batches x 2 head-halves. Each core computes QKV
projections + causal attention for 8 heads of one batch plus the partial
output projection for its head-half's columns of Wo.

I/O path (the axon tunnel moves ~30 MB/s, so bytes through it dominate
end-to-end latency):
  - Host uploads x and the weights in f32, SHARDED over the 8 cores with
    no duplication (~48 MB total; full precision is kept end-to-end so
    the only quantization anywhere is the f16 download of the final,
    already-reduced output -- ~5e-4 relative, applied once).
  - A pack jit (XLA, on device) all-gathers within the fast on-device
    interconnect, slices each core's batch / head-half, transposes x
    (so the Bass kernel needs no transpose of x on the PE), and lays
    tensors out exactly as the Bass kernel expects.
  - The Bass NEFF runs on all 8 cores (one jit whose operands are the
    pack jit's outputs, parameters in order).
  - A reduce jit psums the two head-half partials of each batch on
    device, adds the effective bias (bo + bv routed through Wo: softmax
    rows sum to 1, so the V-bias contribution is the constant vector
    bv @ Wo.T), and returns each core's distinct half of the final
    output as float16 (16 MB back).

Kernel numerics: all matmuls run in fp32r (full PE rate at >=256 moving
rows, ~1e-4 relative rounding); fp32r tensors are DMA'd directly from
DRAM (no on-chip conversion copies). Accumulation is fp32 in PSUM. V is
augmented with a ones column so the ctx matmul's extra output row
accumulates the softmax denominator exactly.
"""
import numpy as np

B, S, E = 4, 2048, 1024
H, D = 16, 64
NP = 4     # head-pairs per core (2 heads packed in the transposed projections)
KT = 8     # E / 128 contraction tiles
NQB = 4    # q blocks of 512
NTT = 16   # t tiles of 128

_NC = None
_RUNNERS = None


def _build(reps=1):
    import concourse.bacc as bacc
    import concourse.tile as tile
    from concourse import mybir
    from concourse.masks import make_identity

    f32, f32r = mybir.dt.float32, mybir.dt.float32r
    f16 = mybir.dt.float16
    Act = mybir.ActivationFunctionType

    nc = bacc.Bacc("TRN2")
    XT = nc.dram_tensor("xt", [E, S], f32r, kind="ExternalInput")
    WQ = nc.dram_tensor("wq", [NP, KT, 128, 128], f32r, kind="ExternalInput")
    WK = nc.dram_tensor("wk", [NP, KT, 128, 128], f32r, kind="ExternalInput")
    WV = nc.dram_tensor("wv", [NP, KT, 128, 128], f32r, kind="ExternalInput")
    BQ = nc.dram_tensor("bq", [NP, 128, 1], f32, kind="ExternalInput")
    BK = nc.dram_tensor("bk", [NP, 128, 1], f32, kind="ExternalInput")
    WO = nc.dram_tensor("wo", [NP, 128, E], f32r, kind="ExternalInput")
    TRI = nc.dram_tensor("tri", [128, 128], f32r, kind="ExternalInput")
    OUT = nc.dram_tensor("out", [S, E], f32, kind="ExternalOutput")

    with tile.TileContext(nc) as tc:
      for _rep in range(reps):
        with tc.tile_pool(name="persist", bufs=1) as pers:
            ident_f = pers.tile([128, 128], f32)
            make_identity(nc, ident_f)
            ident_r = pers.tile([128, 128], f32r)
            nc.vector.tensor_copy(ident_r, ident_f)
            ones16 = pers.tile([128, NTT, 1], f32)
            nc.vector.memset(ones16, 1.0)
            onesrow_f = pers.tile([1, 64], f32)
            nc.vector.memset(onesrow_f, 1.0)
            ones_row = pers.tile([1, 64], f32r)
            nc.vector.tensor_copy(ones_row, onesrow_f)
            tri_r = pers.tile([128, 128], f32r)
            nc.sync.dma_start(tri_r, TRI.ap())
            zeros_f = pers.tile([128, 384], f32)
            nc.vector.memset(zeros_f, 0.0)
            zeros_r = pers.tile([128, 384], f32r)
            nc.vector.tensor_copy(zeros_r, zeros_f)

            bq_t, bk_t = [], []
            for p in range(NP):
                t1 = pers.tile([128, 1], f32, name=f"bq_t{p}")
                nc.sync.dma_start(t1, BQ.ap()[p])
                bq_t.append(t1)
                t2 = pers.tile([128, 1], f32, name=f"bk_t{p}")
                nc.sync.dma_start(t2, BK.ap()[p])
                bk_t.append(t2)

            with tc.tile_pool(name="ctxp", bufs=1) as ctxp:
                ctxN = [ctxp.tile([128, S], f32r, name=f"ctxN{i}") for i in range(NP)]

                with tc.tile_pool(name="xtp", bufs=1) as xtp:
                    xT = [xtp.tile([128, S], f32r, name=f"xT{i}") for i in range(KT)]

                    # ---- Phase A: stream weights (pair 0 first) and x^T.
                    # x^T is pre-transposed on device by the pack jit; the
                    # first half-columns of every k tile land first so the
                    # first projection chains can start early. Weight tiles
                    # come from double-buffered pools (one-pair prefetch).
                    with tc.tile_pool(name="wpool", bufs=2) as wp:
                        def _wtiles(p):
                            out = []
                            for nm in ("q", "k", "v"):
                                out.append([wp.tile([128, 128], f32r,
                                                    name=f"w{nm}_{k}")
                                            for k in range(KT)])
                            return out

                        wq_0, wk_0, wv_0 = _wtiles(0)
                        for k in range(KT):
                            nc.sync.dma_start(wq_0[k], WQ.ap()[0, k])
                        for half in range(2):
                            for k in range(KT):
                                nc.sync.dma_start(
                                    xT[k][:, half * 1024:(half + 1) * 1024],
                                    XT.ap()[k * 128:(k + 1) * 128,
                                            half * 1024:(half + 1) * 1024])
                            if half == 0:
                                for k in range(KT):
                                    nc.sync.dma_start(wk_0[k], WK.ap()[0, k])
                                for k in range(KT):
                                    nc.sync.dma_start(wv_0[k], WV.ap()[0, k])

                        # ---- Phases B+C: per pair, QKV projection then attention ----
                        with tc.tile_pool(name="qtp", bufs=2) as qtp, \
                             tc.tile_pool(name="ktp", bufs=2) as ktp, \
                             tc.tile_pool(name="vnp", bufs=2) as vnp, \
                             tc.tile_pool(name="vt2", bufs=1) as vt2p, \
                             tc.tile_pool(name="expp", bufs=5) as expp, \
                             tc.tile_pool(name="rp", bufs=4) as rp, \
                             tc.tile_pool(name="psB", bufs=4, space="PSUM") as pB, \
                             tc.tile_pool(name="psCTX", bufs=1, space="PSUM") as psCTX:
                          for p in range(NP):
                            if p == 0:
                                wq_p, wk_p, wv_p = wq_0, wk_0, wv_0
                            else:
                                wq_p, wk_p, wv_p = _wtiles(p)
                                for W_, dst in ((WQ, wq_p), (WK, wk_p), (WV, wv_p)):
                                    for k in range(KT):
                                        nc.sync.dma_start(dst[k], W_.ap()[p, k])
                            qt = qtp.tile([128, S], f32r, name="qt")
                            kt = ktp.tile([128, S], f32r, name="kt")
                            vn = vnp.tile([128, 2, NTT, 65], f32r, name="vn")
                            vt2 = vt2p.tile([128, S], f32r)

                            # QKV projections (transposed layout, 2-head packed)
                            for wrs, bias_, dest in (
                                (wq_p, bq_t[p], qt),
                                (wk_p, bk_t[p], kt),
                                (wv_p, None, vt2),
                            ):
                                for half in range(2):
                                    pss = [pB.tile([128, 512], f32, name="pss", bufs=2)
                                           for _ in range(2)]
                                    for k in range(KT):
                                        for i in range(2):
                                            nb = 2 * half + i
                                            nc.tensor.matmul(
                                                pss[i], wrs[k],
                                                xT[k][:, nb * 512:(nb + 1) * 512],
                                                start=(k == 0), stop=(k == KT - 1),
                                            )
                                    for i in range(2):
                                        nb = 2 * half + i
                                        dslc = dest[:, nb * 512:(nb + 1) * 512]
                                        if bias_ is not None:
                                            nc.vector.tensor_scalar_add(dslc, pss[i], bias_)
                                        else:
                                            nc.vector.tensor_copy(dslc, pss[i])
                            # V back to natural [t, d] layout, split per head + ones col
                            for h in range(2):
                                nc.vector.tensor_copy(vn[:, h, :, 64:65], ones16)
                            for tt in range(NTT):
                                tp2 = pB.tile([128, 128], f32r, name="sc", bufs=4)
                                nc.tensor.transpose(tp2, vt2[:, tt * 128:(tt + 1) * 128], ident_r)
                                for h in range(2):
                                    nc.vector.tensor_copy(
                                        vn[:, h, tt, 0:64], tp2[:, h * 64:(h + 1) * 64])

                            # attention for this pair
                            for qb in range(NQB):
                                T = 4 * (qb + 1)  # causal: t-tiles 0..T-1
                                cps = [psCTX.tile([65, 512], f32, name=f"cps{h}")
                                       for h in range(2)]
                                prev_exp = None
                                for tt in range(T):
                                    scs = []
                                    for h in range(2):
                                        sc = pB.tile([128, 512], f32, name="sc", bufs=4)
                                        nc.tensor.matmul(
                                            sc,
                                            kt[h * 64:(h + 1) * 64, tt * 128:(tt + 1) * 128],
                                            qt[h * 64:(h + 1) * 64, qb * 512:(qb + 1) * 512],
                                            start=True, stop=True,
                                        )
                                        scs.append(sc)
                                    if prev_exp is not None:
                                        for h in range(2):
                                            nc.tensor.matmul(
                                                cps[h], vn[:, h, tt - 1, :], prev_exp[h],
                                                start=(tt - 1 == 0), stop=False,
                                            )
                                    j = tt - 4 * qb  # >=0 on diagonal tiles
                                    cur = []
                                    for h in range(2):
                                        ex = expp.tile([128, 512], f32r)
                                        if j >= 1:
                                            nc.gpsimd.tensor_copy(
                                                ex[:, 0:j * 128], zeros_r[:, 0:j * 128])
                                        if j >= 0:
                                            nc.scalar.activation(
                                                ex[:, j * 128:512], scs[h][:, j * 128:512],
                                                Act.Exp, scale=0.125)
                                            nc.vector.tensor_mul(
                                                ex[:, j * 128:(j + 1) * 128],
                                                ex[:, j * 128:(j + 1) * 128], tri_r)
                                        else:
                                            nc.scalar.activation(ex, scs[h], Act.Exp, scale=0.125)
                                        cur.append(ex)
                                    prev_exp = cur
                                for h in range(2):
                                    nc.tensor.matmul(
                                        cps[h], vn[:, h, T - 1, :], prev_exp[h],
                                        start=(T - 1 == 0), stop=True,
                                    )
                                # evict cps to SBUF fast (frees PSUM banks), then
                                # denominators (row 64) -> bcast -> reciprocal -> normalize
                                for h in range(2):
                                    csb = rp.tile([65, 512], f32, name="csb", bufs=2)
                                    nc.vector.tensor_copy(csb, cps[h])
                                    rh = rp.tile([1, 512], f32r, name="rh")
                                    nc.vector.tensor_copy(rh, csb[64:65, :])
                                    rb = pB.tile([64, 512], f32, name="sc", bufs=4)
                                    nc.tensor.matmul(rb, ones_row, rh, start=True, stop=True)
                                    rbs = rp.tile([64, 512], f32, name="rbs", bufs=2)
                                    nc.vector.reciprocal(rbs, rb)
                                    nc.vector.tensor_mul(
                                        ctxN[p][h * 64:(h + 1) * 64, qb * 512:(qb + 1) * 512],
                                        csb[0:64, :], rbs,
                                    )

                # ---- Phase D: output projection (partial, this head-half).
                # (xT pool is closed here, freeing SBUF for the Wo tiles.)
                with tc.tile_pool(name="stD", bufs=2) as sd, \
                     tc.tile_pool(name="wo2", bufs=1) as wop, \
                     tc.tile_pool(name="psD", bufs=4, space="PSUM") as pD:
                    wo_r = []
                    for p in range(NP):
                        wr2 = wop.tile([128, E], f32r, name=f"wo2_{p}")
                        nc.sync.dma_start(wr2, WO.ap()[p])
                        wo_r.append(wr2)
                    for qt_i in range(NTT):
                        ob = sd.tile([128, E], f32, name="ob")
                        for eh in range(2):
                            ps = pD.tile([128, 512], f32, name="psd")
                            for p in range(NP):
                                nc.tensor.matmul(
                                    ps,
                                    ctxN[p][:, qt_i * 128:(qt_i + 1) * 128],
                                    wo_r[p][:, eh * 512:(eh + 1) * 512],
                                    start=(p == 0), stop=(p == NP - 1),
                                )
                            nc.vector.tensor_copy(ob[:, eh * 512:(eh + 1) * 512], ps)
                            nc.sync.dma_start(
                                OUT.ap()[qt_i * 128:(qt_i + 1) * 128,
                                         eh * 512:(eh + 1) * 512],
                                ob[:, eh * 512:(eh + 1) * 512])

    nc.finalize()
    return nc


def _get_nc():
    global _NC
    if _NC is None:
        _NC = _build()
    return _NC


def _pack_w(Wh):
    # [8, E, D] -> [NP, KT, 128, 128]; out[p,k,i,j] = Wh[2p + j//64, k*128+i, j%64]
    w = Wh.reshape(NP, 2, E, D)
    w = np.transpose(w, (0, 2, 1, 3)).reshape(NP, E, 128)
    w = w.reshape(NP, KT, 128, 128)
    return np.ascontiguousarray(w, dtype=np.float32)


def build_in_maps(inputs):
    """Host-side per-core Bass input maps (used by test benches; kernel()
    itself packs on device)."""
    x = np.asarray(inputs["x"], np.float32)
    Wq, bq = np.asarray(inputs["Wq"], np.float32), np.asarray(inputs["bq"], np.float32)
    Wk, bk = np.asarray(inputs["Wk"], np.float32), np.asarray(inputs["bk"], np.float32)
    Wv = np.asarray(inputs["Wv"], np.float32)
    Wo = np.asarray(inputs["Wo"], np.float32)
    tri = (np.arange(128)[None, :] >= np.arange(128)[:, None]).astype(np.float32)
    in_maps = []
    for c in range(8):
        b, hh = divmod(c, 2)
        hsel = slice(hh * 8, hh * 8 + 8)
        in_maps.append({
            "xt": np.ascontiguousarray(x[b].T),
            "wq": _pack_w(Wq[hsel]),
            "wk": _pack_w(Wk[hsel]),
            "wv": _pack_w(Wv[hsel]),
            "bq": np.ascontiguousarray(bq[hsel].reshape(NP, 128, 1)),
            "bk": np.ascontiguousarray(bk[hsel].reshape(NP, 128, 1)),
            "wo": np.ascontiguousarray(
                Wo[:, hh * 512:(hh + 1) * 512].T.reshape(NP, 128, E),
                dtype=np.float32),
            "tri": np.ascontiguousarray(tri),
        })
    return in_maps


def _make_bass_jit(nc, n_cores):
    import jax
    from jax.sharding import Mesh, PartitionSpec
    from jax.experimental.shard_map import shard_map
    from concourse import mybir
    from concourse.bass2jax import (
        _bass_exec_p, install_neuronx_cc_hook, partition_id_tensor)

    install_neuronx_cc_hook()
    partition_name = nc.partition_id_tensor.name if nc.partition_id_tensor else None

    in_names, out_names, out_avals = [], [], []
    for alloc in nc.m.functions[0].allocations:
        if not isinstance(alloc, mybir.MemoryLocationSet):
            continue
        name = alloc.memorylocations[0].name
        if alloc.kind == "ExternalInput":
            if name != partition_name:
                in_names.append(name)
        elif alloc.kind == "ExternalOutput":
            out_names.append(name)
            out_avals.append(jax.core.ShapedArray(
                tuple(alloc.tensor_shape), mybir.dt.np(alloc.dtype)))

    n_params = len(in_names)
    all_names = list(in_names) + list(out_names)
    if partition_name is not None:
        all_names.append(partition_name)

    def _body(*args):
        operands = list(args)
        if partition_name is not None:
            operands.append(partition_id_tensor())
        return tuple(_bass_exec_p.bind(
            *operands,
            out_avals=tuple(out_avals),
            in_names=tuple(all_names),
            out_names=tuple(out_names),
            lowering_input_output_aliases=(),
            sim_require_finite=False,
            sim_require_nnan=False,
            nc=nc,
        ))

    devices = jax.devices()[:n_cores]
    mesh = Mesh(np.asarray(devices), ("core",))
    specs_in = (PartitionSpec("core"),) * (n_params + len(out_names))
    specs_out = (PartitionSpec("core"),) * len(out_names)
    jitted = jax.jit(
        shard_map(_body, mesh=mesh, in_specs=specs_in, out_specs=specs_out,
                  check_rep=False),
        donate_argnums=tuple(range(n_params, n_params + len(out_names))),
    )
    return jitted, in_names, out_names, mesh


def _get_runners():
    global _RUNNERS
    if _RUNNERS is not None:
        return _RUNNERS
    import jax
    import jax.numpy as jnp
    from jax.sharding import Mesh, PartitionSpec, NamedSharding
    from jax.experimental.shard_map import shard_map

    nc = _get_nc()
    bass_jit, in_names, out_names, mesh = _make_bass_jit(nc, 8)
    assert in_names == ["xt", "wq", "wk", "wv", "bq", "bk", "wo", "tri"], in_names
    assert out_names == ["out"], out_names

    P = PartitionSpec

    def _pack(x32, wq32, wk32, wv32, bq32, bk32, wo32):
        # local shards: x32 [1,1024,1024] f32, w*32 [2,1024,64] f32,
        # b*32 [2,64] f32, wo32 [128,1024] f32
        idx = jax.lax.axis_index("core")
        b, hh = idx // 2, idx % 2

        xg = jax.lax.all_gather(x32, "core", axis=0, tiled=True)  # [8,1024,1024]
        xg = xg.reshape(B, S, E)
        xb = jax.lax.dynamic_slice(xg, (b, 0, 0), (1, S, E))[0]
        xt = xb.T  # [E, S]

        def packw(wg16):
            wg = jax.lax.all_gather(wg16, "core", axis=0, tiled=True)  # [16,1024,64]
            wh = jax.lax.dynamic_slice(wg, (hh * 8, 0, 0), (8, E, D))
            w = wh.reshape(NP, 2, E, D).transpose(0, 2, 1, 3)
            return w.reshape(NP, KT, 128, 128)

        wq = packw(wq32)
        wk = packw(wk32)
        wv = packw(wv32)

        def packb(b32):
            bg = jax.lax.all_gather(b32, "core", axis=0, tiled=True)  # [16,64]
            bh = jax.lax.dynamic_slice(bg, (hh * 8, 0), (8, D))
            return bh.reshape(NP, 128, 1)

        bq = packb(bq32)
        bk = packb(bk32)

        wog = jax.lax.all_gather(wo32, "core", axis=0, tiled=True)  # [1024,1024]
        woh = jax.lax.dynamic_slice(wog, (0, hh * 512), (E, 512))
        wo = woh.T.reshape(NP, 128, E)

        col = jax.lax.broadcasted_iota(jnp.int32, (128, 128), 1)
        row = jax.lax.broadcasted_iota(jnp.int32, (128, 128), 0)
        tri = (col >= row).astype(jnp.float32)

        zeros = jnp.zeros((S, E), jnp.float32)
        return xt, wq, wk, wv, bq, bk, wo, tri, zeros

    pack_jit = jax.jit(shard_map(
        _pack, mesh=mesh,
        in_specs=(P("core"),) * 7,
        out_specs=(P("core"),) * 9,
        check_rep=False))

    def _reduce(part, boeff):
        # part [S,E] f32 local partial; boeff [1,E] f32
        idx = jax.lax.axis_index("core")
        hh = idx % 2
        psummed = jax.lax.psum(
            part, "core",
            axis_index_groups=[[0, 1], [2, 3], [4, 5], [6, 7]])
        half = jax.lax.dynamic_slice(
            psummed, (hh * (S // 2), 0), (S // 2, E))
        return (half + boeff).astype(jnp.float16)

    reduce_jit = jax.jit(shard_map(
        _reduce, mesh=mesh,
        in_specs=(P("core"), P("core")),
        out_specs=P("core"),
        check_rep=False))

    sh = NamedSharding(mesh, P("core"))
    _RUNNERS = (pack_jit, bass_jit, reduce_jit, sh)
    return _RUNNERS


def kernel(x, Wq, bq, Wk, bk, Wv, bv, Wo, bo):
    import jax

    x = np.asarray(x, dtype=np.float32)
    Wq = np.asarray(Wq, dtype=np.float32)
    bq = np.asarray(bq, dtype=np.float32)
    Wk = np.asarray(Wk, dtype=np.float32)
    bk = np.asarray(bk, dtype=np.float32)
    Wv = np.asarray(Wv, dtype=np.float32)
    bv = np.asarray(bv, dtype=np.float32)
    Wo = np.asarray(Wo, dtype=np.float32)
    bo = np.asarray(bo, dtype=np.float32)

    pack_jit, bass_jit, reduce_jit, sh = _get_runners()

    # effective bias: bo plus bv routed through Wo (softmax rows sum to 1)
    bo_eff = (bo + bv.reshape(-1) @ Wo.T).astype(np.float32)

    put = lambda a: jax.device_put(a, sh)
    gx = put(x.reshape(8, S // 2, E))
    gwq = put(Wq)
    gwk = put(Wk)
    gwv = put(Wv)
    gbq = put(bq)
    gbk = put(bk)
    gwo = put(Wo)
    gboeff = put(np.broadcast_to(bo_eff, (8, E)).copy())

    packed = pack_jit(gx, gwq, gwk, gwv, gbq, gbk, gwo)
    parts = bass_jit(*packed)
    out16 = reduce_jit(parts[0], gboeff)
    out = np.asarray(out16).astype(np.float32)
    return out.reshape(B, S, E)
